# revision 1
# baseline (speedup 1.0000x reference)
"""Trainium2 Bass kernel for BlockAttentionResidual.

Reference computation (fp32):
    K      = rmsnorm(V, w)                      # over d
    logits = einsum('d,lbtd->lbt', q, K)
    attn   = softmax(logits, axis=l)
    h      = einsum('lbt,lbtd->btd', attn, V)

Mapping (per NeuronCore, tokens = flattened (b,t) sharded 8 ways):
    logits[l,t] = inv[l,t] * dot(q*w, V[l,t,:]),  inv = rsqrt(mean(V^2)+eps)
    - dot      : DVE tensor_tensor_reduce (fused multiply+reduce, fp32)
    - sum(V^2) : ACT Square with accum_out
    - inv      : exp(-0.5*ln(mean+eps)) -- keeps every ACT function (Square,
                 Ln, Exp, Copy) inside the single `natural_log_exp_and_others`
                 table set, so no ACT table reloads
    - h        : 6 of 8 l-slices on PE as diag(attn_l) @ V_l accumulated in
                 PSUM (fp32 matmuls), last 2 on DVE scalar_tensor_tensor,
                 the first of which also merges PSUM -> SBUF
"""

from contextlib import ExitStack

import numpy as np

import concourse.bass as bass
import concourse.mybir as mybir
import concourse.tile as tile
from concourse import bacc
from concourse.bass_utils import run_bass_kernel_spmd

NCORES = 8
L = 8
B = 4
T = 4096
D = 1024
BT = B * T
TOK = BT // NCORES  # tokens per core
P = 128
NT = TOK // P  # token tiles per core
HALF = 512  # fp32 moving-operand limit per matmul
NPE = 6  # l-slices accumulated on the tensor engine
EPS = 1e-6
F32 = mybir.dt.float32

_CACHE: dict = {}

import os as _os

K_NT = int(_os.environ.get("K_NT", NT))
K_ACT_BCAST = int(_os.environ.get("K_ACT_BCAST", "1"))
K_NPE = int(_os.environ.get("K_NPE", NPE))
K_INPLACE = int(_os.environ.get("K_INPLACE", "1"))
# dot-product realization: "stt" (fused scalar_tensor_tensor + accum),
# "amr" (custom-DVE affine_mul_reduce), "classic" (mult + reduce split
# between DVE and ACT; K_RED_DVE of the 8 reduces stay on DVE)
K_DOT = _os.environ.get("K_DOT", "stt")
K_RED_DVE = int(_os.environ.get("K_RED_DVE", "4"))
K_DEEP = int(_os.environ.get("K_DEEP", "1"))


def _build_nc(nt=None, npe=None, dot=None, mode="full", reps=1, bigdma=False,
              dma_eng=None, diag_eng="dve", deep=None):
    deep = bool(K_DEEP) if deep is None else deep
    nt = K_NT if nt is None else nt
    npe = K_NPE if npe is None else npe
    dot = K_DOT if dot is None else dot
    A = mybir.ActivationFunctionType
    O = mybir.AluOpType
    X = mybir.AxisListType.X

    nc = bacc.Bacc(
        "TRN2",
        target_bir_lowering=False,
        debug=False,
        enable_asserts=False,
        num_devices=NCORES,
    )
    v_d = nc.dram_tensor("v", [L, TOK, D], F32, kind="ExternalInput")
    qwb_d = nc.dram_tensor("qwb", [P, D], F32, kind="ExternalInput")
    id_d = nc.dram_tensor("ident", [P, P], F32, kind="ExternalInput")
    h_d = nc.dram_tensor("h", [TOK, D], F32, kind="ExternalOutput")

    with tile.TileContext(nc) as tc, ExitStack() as ctx:
        deep = int(deep)
        vb, sb, db, hb, pb = {
            0: (2, 2, 3, 2, 2),
            1: (3, 3, 4, 3, 3),
            2: (4, 4, 6, 3, 4),
        }[min(deep, 2)]
        cpool = ctx.enter_context(tc.tile_pool(name="const", bufs=1))
        vpool = ctx.enter_context(tc.tile_pool(name="vin", bufs=vb))
        spool = ctx.enter_context(tc.tile_pool(name="small", bufs=sb))
        jpool = ctx.enter_context(tc.tile_pool(name="scratch", bufs=1))
        dpool = ctx.enter_context(tc.tile_pool(name="diag", bufs=db))
        hpool = ctx.enter_context(tc.tile_pool(name="hout", bufs=hb))
        ppool = ctx.enter_context(
            tc.tile_pool(name="psum", bufs=pb, space=bass.MemorySpace.PSUM)
        )

        qwb = cpool.tile([P, D], F32, tag="qwb")
        ident = cpool.tile([P, P], F32, tag="ident")
        nc.sync.dma_start(qwb[:], qwb_d[:])
        nc.sync.dma_start(ident[:], id_d[:])

        # stride-0 sinks for the full-size primary outputs of the fused
        # reduce ops (only the accum_out is consumed)
        if K_ACT_BCAST:
            jact = jpool.tile([P, 1], F32, tag="jact")
            jact_out = jact.broadcast_to((P, D))
        else:
            jact = jpool.tile([P, D], F32, tag="jact")
            jact_out = jact[:]
        jvec_bufs = 2 if dot == "classic" else 1

        # per-partition bias constants (no global const-AP registry here)
        zero_b = cpool.tile([P, 1], F32, tag="zero_b")
        eps_b = cpool.tile([P, 1], F32, tag="eps_b")
        nc.vector.memset(zero_b[:], 0.0)
        nc.vector.memset(eps_b[:], EPS)

        for rep_i in range(reps * nt):
            i = rep_i % nt
            if bigdma:
                vta = vpool.tile([P, L, D], F32, tag="vta", name="vta")
                nc.sync.dma_start(
                    vta[:],
                    v_d[:, i * P : (i + 1) * P, :].rearrange("l t d -> t l d"),
                )
                vt = [vta[:, l, :] for l in range(L)]
            else:
                vt = [
                    vpool.tile([P, D], F32, tag=f"v{l}", name=f"v{l}")
                    for l in range(L)
                ]
                eng = nc.sync if dma_eng is None else getattr(nc, dma_eng)
                for l in range(L):
                    eng.dma_start(vt[l][:], v_d[l, i * P : (i + 1) * P, :])

            if mode == "dmaonly":
                hs = hpool.tile([P, D], F32, tag="hs")
                nc.vector.tensor_copy(hs[:], vt[0][:])
                nc.sync.dma_start(h_d[i * P : (i + 1) * P, :], hs[:])
                continue
            ssq = spool.tile([P, L], F32, tag="ssq")
            dotv = spool.tile([P, L], F32, tag="dotv")
            if mode == "nodot":
                nc.vector.memset(dotv[:], 1.0)
            for l in range(L):
                nc.scalar.activation(
                    jact_out,
                    vt[l][:],
                    A.Square,
                    bias=zero_b[:],
                    accum_out=ssq[:, l : l + 1],
                )
                if mode == "nodot":
                    pass
                elif dot == "stt":
                    jvec = jpool.tile([P, D], F32, tag="jvec", bufs=1, name="jvec")
                    nc.vector.scalar_tensor_tensor(
                        jvec[:], vt[l][:], 1.0, qwb[:], O.mult, O.mult,
                        accum_out=dotv[:, l : l + 1],
                    )
                elif dot == "amr":
                    jvec = jpool.tile([P, D], F32, tag="jvec", bufs=1, name="jvec")
                    nc.vector.affine_mul_reduce(
                        jvec[:], dotv[:, l : l + 1], vt[l][:], qwb[:], 1.0, 0.0
                    )
                else:
                    # classic: DVE multiply, reduce split between DVE and ACT
                    jvec = jpool.tile(
                        [P, D], F32, tag="jvec", bufs=jvec_bufs, name="jvec"
                    )
                    nc.vector.tensor_mul(jvec[:], vt[l][:], qwb[:])
                    if l < K_RED_DVE:
                        nc.vector.tensor_reduce(
                            dotv[:, l : l + 1], jvec[:], X, O.add
                        )
                    else:
                        nc.scalar.activation(
                            jact_out, jvec[:], A.Copy,
                            accum_out=dotv[:, l : l + 1],
                        )

            # inv = rsqrt(mean + eps) = exp(-0.5 * ln(ssq/D + eps))
            lnm = spool.tile([P, L], F32, tag="lnm")
            nc.scalar.activation(lnm[:], ssq[:], A.Ln, scale=1.0 / D, bias=eps_b[:])
            inv = spool.tile([P, L], F32, tag="inv")
            nc.scalar.activation(inv[:], lnm[:], A.Exp, scale=-0.5, bias=zero_b[:])

            logits = spool.tile([P, L], F32, tag="logits")
            nc.vector.tensor_mul(logits[:], dotv[:], inv[:])
            nm = spool.tile([P, 1], F32, tag="nm")
            nc.vector.tensor_reduce(nm[:], logits[:], X, O.max, negate=True)
            e = spool.tile([P, L], F32, tag="e")
            s = spool.tile([P, 1], F32, tag="s")
            nc.scalar.activation(e[:], logits[:], A.Exp, bias=nm[:], accum_out=s[:])
            r = spool.tile([P, 1], F32, tag="r")
            nc.vector.reciprocal(r[:], s[:])
            fold_r = npe >= L and mode == "full"
            if fold_r:
                # unnormalized weights feed the diag matmuls; 1/s is applied
                # in the PSUM->SBUF copy below
                attn = e
            else:
                attn = spool.tile([P, L], F32, tag="attn")
                nc.vector.tensor_scalar_mul(attn[:], e[:], r[:])

            # h = sum_l attn_l * V_l : l < NPE via diag(attn_l) matmuls into
            # PSUM, remaining l on DVE
            if npe > 0:
                hp = ppool.tile([P, D], F32, tag="hp")
                for l in range(npe):
                    dg = dpool.tile([P, P], F32, tag="dg")
                    if diag_eng == "act":
                        nc.scalar.mul(dg[:], ident[:], attn[:, l : l + 1])
                    elif not fold_r:
                        # (ident * e_l) * r in one op -- diags don't wait on
                        # the attn tensor, only on e and r
                        nc.vector.tensor_scalar(
                            dg[:], ident[:], e[:, l : l + 1], r[:],
                            O.mult, O.mult,
                        )
                    else:
                        nc.vector.tensor_scalar_mul(
                            dg[:], ident[:], attn[:, l : l + 1]
                        )
                    for h_ in range(2):
                        nc.tensor.matmul(
                            hp[:, h_ * HALF : (h_ + 1) * HALF],
                            dg[:],
                            vt[l][:, h_ * HALF : (h_ + 1) * HALF],
                            start=(l == 0),
                            stop=(l == npe - 1),
                        )
                hs = hpool.tile([P, D], F32, tag="hs")
                if npe >= L:
                    if fold_r:
                        nc.scalar.mul(hs[:], hp[:], r[:])
                    else:
                        nc.scalar.copy(hs[:], hp[:])
                    rest = range(L, L)
                else:
                    nc.vector.scalar_tensor_tensor(
                        hs[:], vt[npe][:], attn[:, npe : npe + 1], hp[:],
                        O.mult, O.add,
                    )
                    rest = range(npe + 1, L)
            else:
                hs = hpool.tile([P, D], F32, tag="hs")
                nc.vector.tensor_scalar_mul(hs[:], vt[0][:], attn[:, 0:1])
                rest = range(1, L)
            for l in rest:
                if K_INPLACE:
                    nc.vector.scalar_tensor_tensor(
                        hs[:], vt[l][:], attn[:, l : l + 1], hs[:], O.mult, O.add
                    )
                else:
                    hs2 = hpool.tile([P, D], F32, tag="hs", name="hs2")
                    nc.vector.scalar_tensor_tensor(
                        hs2[:], vt[l][:], attn[:, l : l + 1], hs[:], O.mult, O.add
                    )
                    hs = hs2
            nc.sync.dma_start(h_d[i * P : (i + 1) * P, :], hs[:])

    nc.compile()
    return nc


def get_nc():
    if "nc" not in _CACHE:
        _CACHE["nc"] = _build_nc()
    return _CACHE["nc"]


def build_variant(**kw):
    return _build_nc(**kw)


def make_in_maps(blocks, query, norm_weight):
    qw = (query * norm_weight).astype(np.float32)
    qwb = np.ascontiguousarray(np.broadcast_to(qw, (P, D)))
    ident = np.eye(P, dtype=np.float32)
    vr = blocks.reshape(L, BT, D)
    return [
        {
            "v": np.ascontiguousarray(vr[:, c * TOK : (c + 1) * TOK, :]),
            "qwb": qwb,
            "ident": ident,
        }
        for c in range(NCORES)
    ]


def kernel(blocks, query, norm_weight):
    import time

    blocks = np.asarray(blocks, dtype=np.float32)
    query = np.asarray(query, dtype=np.float32)
    norm_weight = np.asarray(norm_weight, dtype=np.float32)
    nc = get_nc()
    in_maps = make_in_maps(blocks, query, norm_weight)
    last_exc = None
    for attempt in range(3):
        try:
            res = run_bass_kernel_spmd(nc, in_maps, core_ids=list(range(NCORES)))
            break
        except Exception as exc:  # transient device-wedge after a prior crash
            last_exc = exc
            time.sleep(45)
    else:
        raise last_exc
    h = np.concatenate([res.results[c]["h"] for c in range(NCORES)], axis=0)
    return h.reshape(B, T, D)



# revision 39
# speedup vs baseline: 145.2047x; 145.2047x over previous
"""Trainium2 Bass kernel for BlockAttentionResidual.

Reference computation (fp32):
    K      = rmsnorm(V, w)                      # over d
    logits = einsum('d,lbtd->lbt', q, K)
    attn   = softmax(logits, axis=l)
    h      = einsum('lbt,lbtd->btd', attn, V)

Mapping (per NeuronCore, tokens = flattened (b,t) sharded 8 ways), v2:
    - ONE rearranged [t, l, d] input DMA per 128-token tile (4 MB, 4 KB
      descriptors) instead of 8 per-l DMAs: the HWDGE/SP sequencer cost
      (~0.65-2 us per dma_start) dominated v1.
    - ssq     : ACT Square with accum_out (some slices on GPSIMD stt)
    - dot     : fp32 scalar_tensor_tensor with accum_out, split between
                DVE and GPSIMD (both engines' stt reduce in fp32)
    - inv     : exp(-0.5*ln(mean+eps)); the Bacc act-table pass is pinned
                to `natural_log_exp_and_others` so Square/Ln/Exp/Copy all
                resolve to ONE table set (v1 thrashed 2 table loads/iter)
    - h       : all 8 l-slices as diag(e_l) @ V_l on PE in float32r
                (1 cycle/row vs 4 for plain fp32), accumulated in PSUM;
                1/sum folded into the ACT PSUM->SBUF copy
    - output  : bf16 (halves the store traffic; upcast on host)
"""

from contextlib import ExitStack

import numpy as np

import bass_rust as _bass_rust
import concourse.bass as bass
import concourse.mybir as mybir
import concourse.tile as tile
from concourse import bacc
from concourse.bass_utils import run_bass_kernel_spmd
from concourse.hw_specs import get_activation_tables

NCORES = 8
L = 8
B = 4
T = 4096
D = 1024
BT = B * T
TOK = BT // NCORES  # tokens per core
P = 128
NT = TOK // P  # token tiles per core
HALF = 512  # one fp32 PSUM bank of output columns per matmul
EPS = 1e-6
F32 = mybir.dt.float32
F32R = mybir.dt.float32r
BF16 = mybir.dt.bfloat16

_CACHE: dict = {}

import os as _os

K_DOT_POOL = int(_os.environ.get("K_DOT_POOL", "0"))  # dot slices on gpsimd
K_SQ_POOL = int(_os.environ.get("K_SQ_POOL", "0"))  # square slices on gpsimd
K_SQ_DVE = int(_os.environ.get("K_SQ_DVE", "0"))  # square slices on DVE
K_DIAG_ONE = int(_os.environ.get("K_DIAG_ONE", "0"))  # all diags in one DVE op
K_PE_WARM = int(_os.environ.get("K_PE_WARM", "0"))  # keep-warm dummy matmuls
K_CLAMP_SMAX = int(_os.environ.get("K_CLAMP_SMAX", "0"))  # clamp+shift softmax
K_DMA_SPLIT = int(_os.environ.get("K_DMA_SPLIT", "1"))  # input DMAs per tile
K_OUT_BF16 = int(_os.environ.get("K_OUT_BF16", "1"))
K_DEEP = int(_os.environ.get("K_DEEP", "5"))  # input-tile pipeline depth
K_OUT_GROUP = int(_os.environ.get("K_OUT_GROUP", "2"))  # tiles per output DMA
K_PIN_ACT = int(_os.environ.get("K_PIN_ACT", "1"))
K_F32R = int(_os.environ.get("K_F32R", "2"))
K_NPE = int(_os.environ.get("K_NPE", str(L)))  # l-slices on the tensor engine
K_SKEW2 = int(_os.environ.get("K_SKEW2", "1"))  # extra stage skew for diag+MM


class _PinnedBacc(bacc.Bacc):
    """Bacc whose act-table pass only considers one table set.

    The rust pass greedily picks the first table containing each
    activation's function, which flips between `exp_and_others` (square,
    exp, copy) and `natural_log` (ln) every iteration -- 2 table loads
    (~2.7 us each on HW) per tile.  Emptying every other set (positions
    preserved, so act_func_set_id still indexes act_info.json correctly)
    forces the single superset table and lets the fixpoint hoist one load
    into the preamble.
    """

    ACT_KEEP = "natural_log_exp_and_others"

    def insert_act_table_loads(self):
        if not K_PIN_ACT:
            return super().insert_act_table_loads()
        has_activation = any(
            isinstance(i, mybir.InstActivation)
            for b in self.main_func.blocks
            for i in b.instructions
        )
        if not has_activation:
            return
        tables = [
            (name, (funcs if name == self.ACT_KEEP else set()))
            for name, funcs in get_activation_tables(self.m.arch).items()
        ]
        assert any(funcs for _, funcs in tables), "pinned act table missing"
        _bass_rust.insert_act_table_loads(self, tables)


def _build_nc(nt=None, reps=1, dot_pool=None, sq_pool=None, dma_split=None,
              out_bf16=None, deep=None, npe=None, f32r=None, out_group=None,
              mode="full", sbufs=4, pbufs=3, hbufs=3, dbufs=12, order="mel",
              sq_dve=None, skew2=None, diag_one=None, pe_warm=None, clamp_smax=None):
    nt = NT if nt is None else nt
    dot_pool = K_DOT_POOL if dot_pool is None else dot_pool
    sq_pool = K_SQ_POOL if sq_pool is None else sq_pool
    dma_split = K_DMA_SPLIT if dma_split is None else dma_split
    out_bf16 = K_OUT_BF16 if out_bf16 is None else out_bf16
    deep = K_DEEP if deep is None else deep
    npe = K_NPE if npe is None else npe
    f32r = K_F32R if f32r is None else f32r
    out_group = K_OUT_GROUP if out_group is None else out_group
    sq_dve = K_SQ_DVE if sq_dve is None else sq_dve
    diag_one = K_DIAG_ONE if diag_one is None else diag_one
    pe_warm = K_PE_WARM if pe_warm is None else pe_warm
    clamp_smax = K_CLAMP_SMAX if clamp_smax is None else clamp_smax
    skew2 = K_SKEW2 if skew2 is None else skew2
    if diag_one:
        dbufs = min(dbufs, 3)  # [P, L, P] diag tiles are 4 KB/partition
    assert nt % out_group == 0
    A = mybir.ActivationFunctionType
    O = mybir.AluOpType
    X = mybir.AxisListType.X
    mm_dt = F32R if f32r else F32
    out_dt = BF16 if out_bf16 else F32

    nc = _PinnedBacc(
        "TRN2",
        target_bir_lowering=False,
        debug=False,
        enable_asserts=False,
        num_devices=NCORES,
    )
    v_dt = F32R if f32r == 2 else F32
    v_d = nc.dram_tensor("v", [L, TOK, D], v_dt, kind="ExternalInput")
    qwb_d = nc.dram_tensor("qwb", [P, D], F32, kind="ExternalInput")
    id_d = nc.dram_tensor("ident", [P, P], F32, kind="ExternalInput")
    h_d = nc.dram_tensor("h", [TOK, D], out_dt, kind="ExternalOutput")

    with tile.TileContext(nc) as tc, ExitStack() as ctx:
        cpool = ctx.enter_context(tc.tile_pool(name="const", bufs=1))
        vpool = ctx.enter_context(tc.tile_pool(name="vin", bufs=deep))
        spool = ctx.enter_context(tc.tile_pool(name="small", bufs=sbufs))
        jpool = ctx.enter_context(tc.tile_pool(name="scratch", bufs=1))
        dpool = ctx.enter_context(tc.tile_pool(name="diag", bufs=dbufs))
        hpool = ctx.enter_context(tc.tile_pool(name="hout", bufs=hbufs))
        ppool = ctx.enter_context(
            tc.tile_pool(name="psum", bufs=pbufs, space=bass.MemorySpace.PSUM)
        )
        if pe_warm:
            wpool = ctx.enter_context(
                tc.tile_pool(name="wpsum", bufs=1, space=bass.MemorySpace.PSUM)
            )
            wp = wpool.tile([P, HALF], F32, tag="wp")

        qwb = cpool.tile([P, D], F32, tag="qwb")
        ident = cpool.tile([P, P], F32, tag="ident")
        nc.sync.dma_start(qwb[:], qwb_d[:])
        nc.sync.dma_start(ident[:], id_d[:])

        # stride-0 sink for the ACT Square primary output (only accum_out
        # is consumed); separate full sinks per stt engine so cross-engine
        # WAW hazards don't serialize the dot products
        jact = jpool.tile([P, 1], F32, tag="jact")
        jact_out = jact.broadcast_to((P, D))
        jvec_dve = jpool.tile([P, D], F32, tag="jvec_dve")
        jvec_pool = jpool.tile([P, D], F32, tag="jvec_pool")

        zero_b = cpool.tile([P, 1], F32, tag="zero_b")
        eps_b = cpool.tile([P, 1], F32, tag="eps_b")
        shift_b = cpool.tile([P, 1], F32, tag="shift_b")
        nc.vector.memset(zero_b[:], 0.0)
        nc.vector.memset(eps_b[:], EPS)
        nc.vector.memset(shift_b[:], -50.0)

        def flush(i0, tile_):
            # grouped store on the SWDGE ring (Pool); emitted two groups
            # after the tiles were computed so its semaphore wait is
            # already satisfied and never stalls the Pool sequencer
            nc.gpsimd.dma_start(
                h_d[i0 * P : (i0 + out_group) * P, :].rearrange(
                    "(g t) d -> t g d", g=out_group
                ),
                tile_[:],
            )

        # ---- software pipeline: per emission step j, issue
        #   dma(j) -> early(j-1) -> mid(j-2) -> late(j-3)
        # so every engine's in-order program only ever reaches
        # instructions whose inputs were produced >= 1 full step ago
        # (waits pre-satisfied; no sequencer stalls on late-stage deps).
        st: dict = {}  # per-inflight-tile state
        pending = []
        hg_cur = [None]

        def stage_dma(k):
            i = k % nt
            vta = vpool.tile([P, L, D], v_dt, tag="vta", name="vta")
            nsp = L // dma_split
            for sp in range(dma_split):
                eng = nc.sync if sp % 2 == 0 else nc.scalar
                eng.dma_start(
                    vta[:, sp * nsp : (sp + 1) * nsp, :],
                    v_d[sp * nsp : (sp + 1) * nsp, i * P : (i + 1) * P, :]
                    .rearrange("l t d -> t l d"),
                )
            st[k] = {"vta": vta}

        def vslices(d):
            if v_dt is F32:
                return [d["vta"][:, l, :] for l in range(L)]
            return [d["vta"][:, l, :].bitcast(F32) for l in range(L)]

        def stage_early(k):
            d = st[k]
            vt = vslices(d)
            ssq = spool.tile([P, L], F32, tag="ssq")
            dotv = spool.tile([P, L], F32, tag="dotv")
            if "nosq" in mode:
                nc.vector.memset(ssq[:], 1.0)
            else:
                for l in range(L):
                    if l < sq_pool:
                        nc.gpsimd.scalar_tensor_tensor(
                            jvec_pool[:], vt[l], 1.0, vt[l], O.mult, O.mult,
                            accum_out=ssq[:, l : l + 1],
                        )
                    elif l < sq_pool + sq_dve:
                        nc.vector.scalar_tensor_tensor(
                            jvec_dve[:], vt[l], 1.0, vt[l], O.mult, O.mult,
                            accum_out=ssq[:, l : l + 1],
                        )
                    else:
                        nc.scalar.activation(
                            jact_out, vt[l], A.Square,
                            bias=zero_b[:], accum_out=ssq[:, l : l + 1],
                        )
            if "nodot" in mode:
                nc.vector.memset(dotv[:], 1.0)
            else:
                for l in range(L):
                    if l >= L - dot_pool:
                        nc.gpsimd.scalar_tensor_tensor(
                            jvec_pool[:], vt[l], 1.0, qwb[:], O.mult, O.mult,
                            accum_out=dotv[:, l : l + 1],
                        )
                    else:
                        nc.vector.scalar_tensor_tensor(
                            jvec_dve[:], vt[l], 1.0, qwb[:], O.mult, O.mult,
                            accum_out=dotv[:, l : l + 1],
                        )
            d["ssq"], d["dotv"] = ssq, dotv
            # keep the tensor engine's p-state ramped between real bursts
            for _ in range(pe_warm):
                nc.tensor.matmul(
                    wp[:], ident[:].bitcast(mm_dt),
                    qwb[:, :HALF].bitcast(mm_dt), start=True, stop=True,
                )

        def stage_soft(k):
            d = st[k]
            if "nosoft" in mode:
                e = spool.tile([P, L], F32, tag="e")
                r = spool.tile([P, 1], F32, tag="r")
                nc.vector.memset(e[:], 0.125)
                nc.vector.memset(r[:], 1.0)
                d["e"], d["r"] = e, r
                return
            # inv = rsqrt(mean + eps) = exp(-0.5 * ln(ssq/D + eps))
            lnm = spool.tile([P, L], F32, tag="lnm")
            nc.scalar.activation(
                lnm[:], d["ssq"][:], A.Ln, scale=1.0 / D, bias=eps_b[:]
            )
            inv = spool.tile([P, L], F32, tag="inv")
            nc.scalar.activation(inv[:], lnm[:], A.Exp, scale=-0.5, bias=zero_b[:])
            logits = spool.tile([P, L], F32, tag="logits")
            nc.vector.tensor_mul(logits[:], d["dotv"][:], inv[:])
            e = spool.tile([P, L], F32, tag="e")
            s = spool.tile([P, 1], F32, tag="s")
            if clamp_smax:
                # softmax without the per-token max: logits ~ N(0, |qw|~32),
                # so clamp to [-38, 50] and shift by -50; exp stays in
                # [6e-39, 1] and the clamp distorts only P(max<-38) ~ 4e-8
                # of tokens. Removes the max-reduce and makes the Exp bias
                # a constant (no cross-engine dependency).
                lc = spool.tile([P, L], F32, tag="lc")
                nc.vector.tensor_scalar(
                    lc[:], logits[:], 50.0, -38.0, O.min, O.max
                )
                nc.scalar.activation(
                    e[:], lc[:], A.Exp, bias=shift_b[:], accum_out=s[:]
                )
            else:
                nm = spool.tile([P, 1], F32, tag="nm")
                nc.vector.tensor_reduce(nm[:], logits[:], X, O.max, negate=True)
                nc.scalar.activation(
                    e[:], logits[:], A.Exp, bias=nm[:], accum_out=s[:]
                )
            r = spool.tile([P, 1], F32, tag="r")
            nc.vector.reciprocal(r[:], s[:])
            d["e"], d["r"] = e, r

        def stage_mm(k):
            d = st[k]
            vt = vslices(d)
            e = d["e"]
            # h = sum_l e_l * V_l via diag(e_l) @ V_l accumulated in PSUM
            # (unnormalized; 1/s applied in the PSUM->SBUF copy)
            hp = ppool.tile([P, D], F32, tag="hp")
            nmm = 1 if "nomm" in mode else npe
            if diag_one:
                # all 8 diags in one DVE op: [P, L, P] = bcast(ident) * bcast(e)
                dga = dpool.tile([P, L, P], F32, tag="dga", name="dga")
                nc.vector.tensor_tensor(
                    dga[:],
                    ident[:].unsqueeze(1).broadcast_to((P, L, P)),
                    e[:].unsqueeze(2).broadcast_to((P, L, P)),
                    O.mult,
                )
                dgs = [dga[:, l, :] for l in range(L)]
            else:
                dgs = []
                for l in range(nmm):
                    dg = dpool.tile([P, P], mm_dt if f32r == 2 else F32, tag="dg")
                    nc.vector.tensor_scalar_mul(dg[:], ident[:], e[:, l : l + 1])
                    dgs.append(dg[:])
            for l in range(nmm):
                mv = d["vta"][:, l, :] if v_dt is F32R else vt[l]
                for h_ in range(2):
                    nc.tensor.matmul(
                        hp[:, h_ * HALF : (h_ + 1) * HALF],
                        dgs[l].bitcast(mm_dt),
                        mv[:, h_ * HALF : (h_ + 1) * HALF].bitcast(mm_dt),
                        start=(l == 0),
                        stop=(l == nmm - 1),
                    )
            if npe < L and "nomm" not in mode:
                hx = hpool.tile([P, D], F32, tag="hx", name="hx")
                nc.vector.scalar_tensor_tensor(
                    hx[:], vt[npe][:], e[:, npe : npe + 1], hp[:], O.mult, O.add
                )
                for l in range(npe + 1, L):
                    nc.vector.scalar_tensor_tensor(
                        hx[:], vt[l][:], e[:, l : l + 1], hx[:], O.mult, O.add
                    )
                d["hx"] = hx
            d["hp"] = hp

        def stage_late(k):
            d = st.pop(k)
            i = k % nt
            g = i % out_group
            if g == 0:
                if len(pending) >= 2:
                    flush(*pending.pop(0))
                hg_cur[0] = hpool.tile(
                    [P, out_group, D], out_dt, tag="hg", name="hg"
                )
            hs = hg_cur[0][:, g, :]
            if npe >= L:
                nc.scalar.mul(hs, d["hp"][:], d["r"][:])
            else:
                nc.vector.tensor_scalar_mul(hs, d["hx"][:], d["r"][:])
            if g == out_group - 1:
                pending.append((i - (out_group - 1), hg_cur[0]))

        total = reps * nt
        if mode == "dmaonly":
            for k in range(total + 1):
                if k < total:
                    stage_dma(k)
                if 0 <= k - 1:
                    d = st.pop(k - 1)
                    i = (k - 1) % nt
                    g = i % out_group
                    if g == 0:
                        if len(pending) >= 2:
                            flush(*pending.pop(0))
                        hg_cur[0] = hpool.tile(
                            [P, out_group, D], out_dt, tag="hg", name="hg"
                        )
                    nc.vector.tensor_copy(hg_cur[0][:, g, :], d["vta"][:, 0, :])
                    if g == out_group - 1:
                        pending.append((i - (out_group - 1), hg_cur[0]))
        else:
            off_mm = 2 + skew2
            off_late = 3 + skew2
            for j in range(total + off_late):
                if j < total:
                    stage_dma(j)
                if order == "eml":
                    if 0 <= j - 1 < total:
                        stage_early(j - 1)
                    if 0 <= j - 2 < total:
                        stage_soft(j - 2)
                    if 0 <= j - off_mm < total:
                        stage_mm(j - off_mm)
                    if 0 <= j - off_late < total:
                        stage_late(j - off_late)
                else:  # "mel": short chains first, bulk work last
                    if 0 <= j - 2 < total:
                        stage_soft(j - 2)
                    if 0 <= j - off_mm < total:
                        stage_mm(j - off_mm)
                    if 0 <= j - off_late < total:
                        stage_late(j - off_late)
                    if 0 <= j - 1 < total:
                        stage_early(j - 1)
        for args in pending:
            flush(*args)

    nc.compile()
    return nc


def get_nc():
    if "nc" not in _CACHE:
        _CACHE["nc"] = _build_nc()
    return _CACHE["nc"]


def build_variant(**kw):
    return _build_nc(**kw)


def make_in_maps(blocks, query, norm_weight):
    qw = (query * norm_weight).astype(np.float32)
    qwb = np.ascontiguousarray(np.broadcast_to(qw, (P, D)))
    ident = np.eye(P, dtype=np.float32)
    vr = blocks.reshape(L, BT, D)
    return [
        {
            "v": np.ascontiguousarray(vr[:, c * TOK : (c + 1) * TOK, :]),
            "qwb": qwb,
            "ident": ident,
        }
        for c in range(NCORES)
    ]


def kernel(blocks, query, norm_weight):
    import time

    blocks = np.asarray(blocks, dtype=np.float32)
    query = np.asarray(query, dtype=np.float32)
    norm_weight = np.asarray(norm_weight, dtype=np.float32)
    nc = get_nc()
    in_maps = make_in_maps(blocks, query, norm_weight)
    last_exc = None
    for attempt in range(3):
        try:
            res = run_bass_kernel_spmd(nc, in_maps, core_ids=list(range(NCORES)))
            break
        except Exception as exc:  # transient device-wedge after a prior crash
            last_exc = exc
            time.sleep(45)
    else:
        raise last_exc
    h = np.concatenate(
        [np.asarray(res.results[c]["h"], dtype=np.float32) for c in range(NCORES)],
        axis=0,
    )
    return h.reshape(B, T, D)
